# revision 6
# baseline (speedup 1.0000x reference)
"""Sliding-window (radius-8, K=17) single-head attention along W — v2.

Full problem: feature/position [2, 128, 64, 256] f32; 1x1 convs Wq/Wk (+bias)
produce q/k; scores over a 17-wide window along W; softmax (zero-padded
windows contribute exp(0)=1 to the denominator); output is the attn-weighted
sum of windows of x = feature + position.

Algebraic restructure vs the straightforward kernel:
  S[w',w] = (Wq x(w)+bq)·(Wk x(w')+bk)/sqrt(C)
          = g(w')·x(w) + alpha(w') + beta(w) + c0       with
    g  = M x + u,  M = Wq^T Wk/sc,  u = Wq^T bk/sc  (beta folded into g)
    alpha(w') = v·x(w'), v = Wk^T bq/sc              (per-KEY -> ACT exp bias)
    c0 = bq·bk/sc  (constant; cancels in softmax, baked into the zero-pad
    denominator term exp(-c0))
  so only ONE projection matmul (g) is needed and no q/k bias evictions.
  A constant shift s0 keeps exp() in fp16 range (softmax-invariant).

Banded scores: only queries [0,136) interact with key chunk 0 (keys 0..127)
and queries [120,256) with chunk 1, so the score/den/value matmuls run on
136-wide blocks instead of full 256x256. Score path and value path run in
fp16 (fp16 matmuls are full rate; ~5e-4 element error).

Sharding: data-parallel over (B, H) — 16 rows of [C=128, W=256] per core.
x = f + p lands directly as fp16 via SWDGE DMA-accumulate (no engine time).
Output is fp16 (host widens to f32); rel err vs f32 reference ~1e-3.
"""

import numpy as np
from contextlib import ExitStack

import concourse.bacc as bacc
import concourse.mybir as mybir
import concourse.tile as tile
from concourse.bass_utils import run_bass_kernel_spmd

B, C, H, W = 2, 128, 64, 256
R = 8
NCORES = 8
ROWS = B * H // NCORES        # 16 (b, h) rows per core
CORES_PER_B = NCORES // B     # 4
F32 = mybir.dt.float32
F16 = mybir.dt.float16
EXP = mybir.ActivationFunctionType.Exp
MULT = mybir.AluOpType.mult
ADD = mybir.AluOpType.add
S0 = 7.0                      # constant score shift (softmax-invariant)
NB = 136                      # banded block width (128 + R)
DMA_ACCUM = True              # x = f + p via SWDGE accumulate-cast DMA
USE_DIVIDE = False            # DVE can't read two PSUM operands in one op


def build_nc():
    nc = bacc.Bacc(trn_type="TRN2")
    f_ext = nc.dram_tensor("feature", [C, ROWS, W], F32, kind="ExternalInput")
    p_ext = nc.dram_tensor("position", [C, ROWS, W], F32, kind="ExternalInput")
    mt_ext = nc.dram_tensor("mt", [C, C], F16, kind="ExternalInput")
    id_ext = nc.dram_tensor("ident", [C, C], F16, kind="ExternalInput")
    ones_ext = nc.dram_tensor("ones", [C, C], F16, kind="ExternalInput")
    v_ext = nc.dram_tensor("vt", [C, 1], F16, kind="ExternalInput")
    u_ext = nc.dram_tensor("ut", [C, 1], F32, kind="ExternalInput")
    band_ext = nc.dram_tensor("band01", [C, 2 * NB], F16, kind="ExternalInput")
    oob_ext = nc.dram_tensor("oob16", [C, W], F16, kind="ExternalInput")
    out_ext = nc.dram_tensor("out", [C, ROWS, W], F16, kind="ExternalOutput")

    with tile.TileContext(nc) as tc, ExitStack() as ctx:
        const = ctx.enter_context(tc.tile_pool(name="const", bufs=1))
        xgp = ctx.enter_context(tc.tile_pool(name="xg", bufs=1))

        # x = f + p, landed as fp16: p (cast) then f (accumulate-cast), in
        # 4-row chunks so compute starts early.
        x_sb = xgp.tile([C, ROWS * W], F16, tag="x")
        CH = 4 * W
        if DMA_ACCUM:
            for c4 in range(ROWS // 4):
                sl = slice(c4 * CH, (c4 + 1) * CH)
                nc.gpsimd.dma_start(x_sb[:, sl], p_ext[:, 4 * c4 : 4 * c4 + 4, :])
                nc.gpsimd.dma_start(
                    x_sb[:, sl], f_ext[:, 4 * c4 : 4 * c4 + 4, :], accum_op=ADD
                )

        def cload(shape, dt, ext, tag):
            t = const.tile(shape, dt, tag=tag, name=tag)
            nc.sync.dma_start(t[:], ext[:])
            return t

        mt_t = cload([C, C], F16, mt_ext, "mt")
        ident = cload([C, C], F16, id_ext, "id")
        ones_t = cload([C, C], F16, ones_ext, "ones")
        v_t = cload([C, 1], F16, v_ext, "v")
        u_t = cload([C, 1], F32, u_ext, "u")
        band_t = cload([C, 2 * NB], F16, band_ext, "band")
        oob_t = cload([C, W], F16, oob_ext, "oob")

        if not DMA_ACCUM:
            inp = ctx.enter_context(tc.tile_pool(name="inp", bufs=2))
            for c4 in range(ROWS // 4):
                sl = slice(c4 * CH, (c4 + 1) * CH)
                ft = inp.tile([C, CH], F32, tag="ft")
                nc.sync.dma_start(ft[:], f_ext[:, 4 * c4 : 4 * c4 + 4, :])
                pt = inp.tile([C, CH], F32, tag="pt")
                nc.sync.dma_start(pt[:], p_ext[:, 4 * c4 : 4 * c4 + 4, :])
                if c4 % 2 == 0:
                    nc.vector.tensor_add(x_sb[:, sl], ft[:], pt[:])
                else:
                    nc.gpsimd.tensor_add(x_sb[:, sl], ft[:], pt[:])

        # touch Exp once so the ACT table loads during the input-DMA ramp
        warm = const.tile([C, 1], F32, tag="warm")
        nc.scalar.activation(warm[:], u_t[:], EXP)

        g_sb = xgp.tile([C, ROWS * W], F16, tag="g")

        attp = ctx.enter_context(tc.tile_pool(name="att", bufs=3))
        sbp = ctx.enter_context(tc.tile_pool(name="sb", bufs=2))
        ps_s = ctx.enter_context(tc.tile_pool(name="ps_s", bufs=2, space="PSUM"))
        ps_dn = ctx.enter_context(tc.tile_pool(name="ps_dn", bufs=2, space="PSUM"))
        ps_xt = ctx.enter_context(tc.tile_pool(name="ps_xt", bufs=2, space="PSUM"))
        ps_o = ctx.enter_context(tc.tile_pool(name="ps_o", bufs=2, space="PSUM"))

        for half in range(2):          # 8-row halves; g computed per half
            r0 = 8 * half
            # g = M x (+u at eviction): one N=512 matmul per row-pair,
            # M^T stationary loaded once per half.
            for pr in range(4):
                r = r0 + 2 * pr
                xsl = slice(r * W, (r + 2) * W)
                g_ps = ps_dn.tile([C, 2 * W], F32, tag="dn")
                nc.tensor.matmul(g_ps[:], mt_t[:], x_sb[:, xsl], start=True, stop=True)
                nc.vector.tensor_scalar_add(g_sb[:, xsl], g_ps[:], u_t[:])

            for pr in range(4):
                r = r0 + 2 * pr
                xt_ps = ps_xt.tile([C, 2 * W], F16, tag="xt")
                den_ps = ps_dn.tile([C, 2 * W], F32, tag="dn")
                out_ps = ps_o.tile([C, 2 * W], F32, tag="out")
                xt_sb = sbp.tile([C, 2 * W], F16, tag="xt")
                atts = []
                for rr in range(2):
                    x_r = x_sb[:, (r + rr) * W : (r + rr + 1) * W]
                    g_r = g_sb[:, (r + rr) * W : (r + rr + 1) * W]
                    o0 = rr * W

                    # x^T chunks (PE transpose; x chunk is the stationary)
                    # and alpha = v^T x piggybacked on the same stationary.
                    s_ps = ps_s.tile([C, 2 * NB + 2], F32, tag="s")
                    nc.tensor.transpose(
                        xt_ps[:, o0 : o0 + 128], x_r[:, 0:128], ident[:]
                    )
                    nc.tensor.matmul(
                        s_ps[:, 2 * NB : 2 * NB + 1], x_r[:, 0:128], v_t[:],
                        start=True, stop=True,
                    )
                    nc.tensor.transpose(
                        xt_ps[:, o0 + 128 : o0 + 2 * 128], x_r[:, 128:256], ident[:]
                    )
                    nc.tensor.matmul(
                        s_ps[:, 2 * NB + 1 : 2 * NB + 2], x_r[:, 128:256], v_t[:],
                        start=True, stop=True,
                    )

                    # banded scores: S^T block per key chunk
                    nc.tensor.matmul(
                        s_ps[:, 0:NB], g_r[:, 0:128], x_r[:, 0:NB],
                        start=True, stop=True,
                    )
                    nc.tensor.matmul(
                        s_ps[:, NB : 2 * NB], g_r[:, 128:256], x_r[:, W - NB : W],
                        start=True, stop=True,
                    )

                    # alpha - s0 to SBUF for the exp bias
                    al_sb = sbp.tile([C, 2], F32, tag="al")
                    nc.vector.tensor_scalar_add(
                        al_sb[:], s_ps[:, 2 * NB : 2 * NB + 2], -S0
                    )

                    att = attp.tile([C, 2 * NB], F16, tag="att")
                    nc.scalar.activation(
                        att[:, 0:NB], s_ps[:, 0:NB], EXP, bias=al_sb[:, 0:1]
                    )
                    nc.scalar.activation(
                        att[:, NB : 2 * NB], s_ps[:, NB : 2 * NB], EXP,
                        bias=al_sb[:, 1:2],
                    )
                    # zero out-of-band entries
                    attm = attp.tile([C, 2 * NB], F16, tag="attm")
                    nc.gpsimd.tensor_mul(attm[:], att[:], band_t[:])
                    atts.append(attm)

                    # denominator (all-partition broadcast via ones matmul);
                    # zero-pad contribution pre-baked in oob16.
                    nc.tensor.matmul(
                        den_ps[:, o0 : o0 + W], ones_t[:], oob_t[:],
                        start=True, stop=False,
                    )
                    nc.tensor.matmul(
                        den_ps[:, o0 : o0 + NB], ones_t[:], attm[:, 0:NB],
                        start=False, stop=False,
                    )
                    nc.tensor.matmul(
                        den_ps[:, o0 + W - NB : o0 + W], ones_t[:], attm[:, NB : 2 * NB],
                        start=False, stop=True,
                    )

                nc.vector.tensor_copy(xt_sb[:], xt_ps[:])

                for rr in range(2):
                    attm = atts[rr]
                    o0 = rr * W
                    # out_u = x @ attU via x^T chunks as stationaries
                    nc.tensor.matmul(
                        out_ps[:, o0 : o0 + NB],
                        xt_sb[:, o0 : o0 + 128], attm[:, 0:NB],
                        start=True, stop=True,
                    )
                    nc.tensor.matmul(
                        out_ps[:, o0 + 120 : o0 + NB],
                        xt_sb[:, o0 + 128 : o0 + 256], attm[:, NB : NB + 16],
                        start=False, stop=True, skip_group_check=True,
                    )
                    nc.tensor.matmul(
                        out_ps[:, o0 + NB : o0 + W],
                        xt_sb[:, o0 + 128 : o0 + 256], attm[:, NB + 16 : 2 * NB],
                        start=True, stop=True,
                    )

                ostage = sbp.tile([C, 2 * W], F16, tag="ost")
                if USE_DIVIDE:
                    nc.vector.tensor_tensor(
                        ostage[:], out_ps[:], den_ps[:], mybir.AluOpType.divide
                    )
                else:
                    rden = sbp.tile([C, 2 * W], F32, tag="rd")
                    nc.vector.reciprocal_approx_fast(out=rden[:], in_=den_ps[:])
                    nc.vector.tensor_tensor(ostage[:], out_ps[:], rden[:], MULT)
                nc.sync.dma_start(out_ext[:, r : r + 2, :], ostage[:])

    nc.compile()
    return nc


def host_consts(Wq, bq, Wk, bk):
    sc = np.float32(np.sqrt(np.float32(C)))
    Wq = Wq.astype(np.float64)
    Wk = Wk.astype(np.float64)
    bq = bq.astype(np.float64)
    bk = bk.astype(np.float64)
    M = (Wq.T @ Wk) / sc
    v = (Wk.T @ bq) / sc
    u = (Wq.T @ bk) / sc
    c0 = float(bq @ bk) / sc

    mt = np.ascontiguousarray(M.T).astype(np.float16)      # lhsT for g = M x
    vt = v.reshape(C, 1).astype(np.float16)
    ut = u.reshape(C, 1).astype(np.float32)
    ident = np.eye(C, dtype=np.float16)
    ones = np.ones((C, C), dtype=np.float16)

    # band01[p, col]: chunk0 cols 0..NB-1 (query w=col, key p),
    # chunk1 cols NB..2NB-1 (query w=120+(col-NB), key 128+p)
    band = np.zeros((C, 2 * NB), dtype=np.float16)
    for pp in range(C):
        for col in range(NB):
            if abs(col - pp) <= R:
                band[pp, col] = 1.0
        for col in range(NB):
            if abs((W - NB + col) - (128 + pp)) <= R:
                band[pp, NB + col] = 1.0

    wgrid = np.arange(W)
    oob_row = np.maximum(0, R - wgrid) + np.maximum(0, wgrid - (W - 1 - R))
    oob16 = np.tile(
        (oob_row * np.exp(-c0 - S0) / C).astype(np.float16), (C, 1)
    )
    return mt, vt, ut, ident, ones, band, oob16


def core_inputs(feature, position, Wq, bq, Wk, bk):
    mt, vt, ut, ident, ones, band, oob16 = host_consts(Wq, bq, Wk, bk)
    in_maps = []
    for i in range(NCORES):
        b = i // CORES_PER_B
        h0 = (i % CORES_PER_B) * ROWS
        in_maps.append(
            {
                "feature": np.ascontiguousarray(
                    feature[b, :, h0 : h0 + ROWS, :], dtype=np.float32
                ),
                "position": np.ascontiguousarray(
                    position[b, :, h0 : h0 + ROWS, :], dtype=np.float32
                ),
                "mt": mt,
                "ident": ident,
                "ones": ones,
                "vt": vt,
                "ut": ut,
                "band01": band,
                "oob16": oob16,
            }
        )
    return in_maps


def kernel(feature, position, Wq, bq, Wk, bk):
    feature = np.asarray(feature, dtype=np.float32)
    position = np.asarray(position, dtype=np.float32)
    Wq = np.asarray(Wq, dtype=np.float32)
    bq = np.asarray(bq, dtype=np.float32)
    Wk = np.asarray(Wk, dtype=np.float32)
    bk = np.asarray(bk, dtype=np.float32)
    in_maps = core_inputs(feature, position, Wq, bq, Wk, bk)
    nc = build_nc()
    res = run_bass_kernel_spmd(nc, in_maps, list(range(NCORES)))
    out = np.empty((B, C, H, W), dtype=np.float32)
    for i in range(NCORES):
        b = i // CORES_PER_B
        h0 = (i % CORES_PER_B) * ROWS
        out[b, :, h0 : h0 + ROWS, :] = res.results[i]["out"].astype(np.float32)
    return out
